# revision 23
# baseline (speedup 1.0000x reference)
"""Trainium2 Bass kernel for nn_MixedAttention (ConvBERT-style mixed attention).

Sharding: data-parallel over (batch=4) x (seq halves=2) = 8 cores.
Each core computes output rows [j*1024, (j+1)*1024) of batch b, core = 2*b + j.
k/v are computed redundantly on both cores of a batch pair (no collectives).

Per-core layout strategy (all SBUF tiles [<=128 partitions, free]):
  xT (hidden on partitions, seq on free) drives every projection matmul;
  x is DMA'd in seq-chunks so compute starts before the full load lands.
  q,k,kc,co,conv_attn live transposed [a, s]; v lives natural [s, a].
  Attention processes HEAD PAIRS: the even head occupies partitions 0:64 and
  the odd head 64:128 of kT/qT, so the two score matmuls (contraction 64) are
  row-tiled (tile_position rows 0/64) and run CONCURRENTLY in the PE array,
  writing the two halves of one [128, 1024] PSUM tile that a single EXP
  activation consumes (scale=1/8 folded in).  ctx.T accumulates per head via
  lhsT=[v_h | ones] so the softmax denominator falls out as row 64;
  normalization happens after a PE transpose back to [s, d] layout.
  Conv branch: depthwise conv as 9 shifted diagonal matmuls on PE, pointwise
  matmul, dynamic span kernel softmax'd per head via a selector matmul, and
  the windowed einsum as 9 shifted multiply-adds on DVE with span weights
  broadcast across head dims by DMA.

All non-attention matmuls (projections, conv branch) are emitted through a
filler queue interleaved into the attention emission so the Tile scheduler
fills the PE gaps of the ACT-bound attention phase.
"""

import sys

for _p in ("/opt/trn_rl_repo",):
    if _p not in sys.path:
        sys.path.insert(0, _p)

import numpy as np
import ml_dtypes

HIDDEN = 768
N_HEADS = 6
HEAD_DIM = 64
ALL_HEAD = 384
K = 9
B, S = 4, 2048
CHUNK = 1024          # seq rows per core
N_CORES = 8
BF16 = ml_dtypes.bfloat16

_COMPILED = {}


def _build_program():
    import concourse.bass as bass
    import concourse.mybir as mybir
    import concourse.tile as tile
    from concourse import bacc
    from concourse.masks import make_identity
    from contextlib import ExitStack
    from collections import deque

    dt = mybir.dt
    Alu = mybir.AluOpType
    Act = mybir.ActivationFunctionType

    nc = bacc.Bacc("TRN2", target_bir_lowering=False, debug=False)

    # ---------------- DRAM I/O (host pre-laid in SBUF layout) ----------------
    def din(name, shape, dtype=dt.bfloat16):
        return nc.dram_tensor(name, list(shape), dtype, kind="ExternalInput").ap()

    x4 = din("x4", [128, 4 * 6 * 512])                # xT full seq, 4 chunks
    xl2 = din("xl2", [128, 2 * 6 * 520])              # xT local chunk +-4, 2 halves
    wq = din("wq", [128, 6 * ALL_HEAD])
    wk = din("wk", [128, 6 * ALL_HEAD])
    wv = din("wv", [128, 6 * ALL_HEAD])
    wco = din("wco", [128, 6 * ALL_HEAD])
    wpw = din("wpw", [128, 6 * ALL_HEAD])
    wck = din("wck", [128, 3 * 54])
    dwv = din("dwv", [128, 6 * K], dt.float32)                     # depthwise taps [c, ct*K+k]
    sel = din("sel", [54, 6])                          # head-sum selector
    bvrow = din("bvrow", [1, ALL_HEAD])
    comask = din("comask", [1, 1032])
    bq = din("bq", [128, 3], dt.float32)
    bk = din("bk", [128, 3], dt.float32)
    convb = din("convb", [128, 3], dt.float32)
    bco = din("bco", [128, 3], dt.float32)
    bck = din("bck", [54, 1], dt.float32)

    out = nc.dram_tensor("out", [128, 8 * 768], dt.float32, kind="ExternalOutput").ap()
    pck_dram = nc.dram_tensor("pck_scratch", [54, CHUNK], dt.bfloat16).ap()

    with tile.TileContext(nc) as tc, ExitStack() as ctx:
        singles = ctx.enter_context(tc.tile_pool(name="singles", bufs=1))
        persist = ctx.enter_context(tc.tile_pool(name="persist", bufs=1))
        work = ctx.enter_context(tc.tile_pool(name="work", bufs=3))

        def load(pool, src, shape, dtype=dt.bfloat16, name=None):
            t = pool.tile(shape, dtype, name=name)
            nc.sync.dma_start(out=t, in_=src)
            return t

        # x first (compute starts on chunk 0), then weights in usage order.
        # x is split into per-chunk DMAs so the first projection matmuls can
        # start as soon as chunk 0 lands instead of waiting for the full load.
        xlsb = singles.tile([128, 2, 6, 520], dt.bfloat16, name="xlsb")
        xsb = singles.tile([128, 4, 6, 512], dt.bfloat16, name="xsb")
        nc.sync.dma_start(out=xsb[:, 0], in_=x4[:, 0:3072])
        wk_sb = load(singles, wk, [128, 6, ALL_HEAD], name="wk_sb")
        nc.sync.dma_start(out=xlsb[:, 0], in_=xl2[:, 0:3120])
        wq_sb = load(singles, wq, [128, 6, ALL_HEAD], name="wq_sb")
        wv_sb = load(singles, wv, [128, 6, ALL_HEAD], name="wv_sb")
        for c in range(1, 4):
            nc.sync.dma_start(out=xsb[:, c], in_=x4[:, c * 3072:(c + 1) * 3072])
        nc.sync.dma_start(out=xlsb[:, 1], in_=xl2[:, 3120:6240])
        dwv_sb = load(singles, dwv, [128, 6, K], dt.float32, name="dwv_sb")
        wco_sb = load(singles, wco, [128, 6, ALL_HEAD], name="wco_sb")
        wpw_sb = load(singles, wpw, [128, 6, ALL_HEAD], name="wpw_sb")
        wck_sb = load(singles, wck, [128, 3, 54], name="wck_sb")
        sel_sb = load(singles, sel, [54, 6], name="sel_sb")
        bq_sb = load(singles, bq, [128, 3], dt.float32, name="bq_sb")
        bk_sb = load(singles, bk, [128, 3], dt.float32, name="bk_sb")
        convb_sb = load(singles, convb, [128, 3], dt.float32, name="convb_sb")
        bco_sb = load(singles, bco, [128, 3], dt.float32, name="bco_sb")
        bck_sb = load(singles, bck, [54, 1], dt.float32, name="bck_sb")

        mask_sb = singles.tile([128, 1032], dt.bfloat16, name="mask_sb")
        nc.gpsimd.dma_start(out=mask_sb, in_=comask.to_broadcast([128, 1032]))
        bvb = singles.tile([128, ALL_HEAD], dt.bfloat16, name="bvb")
        nc.gpsimd.dma_start(out=bvb, in_=bvrow.to_broadcast([128, ALL_HEAD]))

        ident = singles.tile([128, 128], dt.bfloat16, name="ident")
        make_identity(nc, ident)

        # diagonal depthwise matrices built on-device (saves 1.8MB of DMA):
        # dwd_sb[c', ct, k, c] = (c'==c) * dw[ct*128+c', k] = ident * tap scalar
        dwd_sb = singles.tile([128, 6, K, 128], dt.bfloat16, name="dwd_sb")
        for ct in range(6):
            for k in range(K):
                nc.vector.tensor_scalar_mul(
                    dwd_sb[:, ct, k, :], ident, dwv_sb[:, ct, k:k + 1])

        # persistent intermediates
        qT = persist.tile([128, 3, CHUNK], dt.bfloat16, name="qT")
        kT = persist.tile([128, 3, S], dt.bfloat16, name="kT")
        dwT = persist.tile([128, 6, CHUNK], dt.bfloat16, name="dwT")
        kcT = persist.tile([128, 3, CHUNK], dt.bfloat16, name="kcT")
        caT = persist.tile([128, 3, CHUNK], dt.bfloat16, name="caT")
        coT = persist.tile([128, 3, 1032], dt.bfloat16, name="coT")
        vsb = persist.tile([128, 16, 6, 65], dt.bfloat16, name="vsb")
        pck = persist.tile([54, CHUNK], dt.bfloat16, name="pck")
        recipc = persist.tile([128, 8, 6], dt.float32, name="recipc")
        accA = persist.tile([128, 3, CHUNK], dt.bfloat16, name="accA")
        accB = persist.tile([128, 3, CHUNK], dt.bfloat16, name="accB")
        accT = persist.tile([128, 3, CHUNK], dt.bfloat16, name="accT")
        stg = persist.tile([128, 8, 768], dt.float32, name="stg")

        nc.vector.memset(vsb[:, :, :, 64:65], 1.0)

        # PSUM pools: pj ring 2 banks + sc 4 banks + cpe/cpo 2 banks = 8
        pj = ctx.enter_context(tc.tile_pool(name="psum_pj", bufs=1, space="PSUM"))
        pa = ctx.enter_context(tc.tile_pool(name="psum_sc", bufs=1, space="PSUM"))
        pc = ctx.enter_context(tc.tile_pool(name="psum_cp", bufs=1, space="PSUM"))

        # ---------------- emission helpers (filler generators) -------------
        # Fillers are generators yielding every ~3 matmuls, so the list
        # scheduler never places more than ~640ns of filler PE work between
        # two consecutive attention score matmuls.
        def v_tile(st):
            pv = pj.tile([128, 512], dt.float32, tag="pj", bufs=2, name="pv")
            sq, o = st // 4, (st % 4) * 128
            for dh in range(6):
                nc.tensor.matmul(
                    pv[:, 0:ALL_HEAD], xsb[:, sq, dh, o:o + 128],
                    wv_sb[:, dh, :], start=(dh == 0), stop=(dh == 5))
                if dh == 2:
                    yield
            nc.vector.tensor_add(
                vsb[:, st, :, 0:64],
                pv[:, 0:ALL_HEAD].rearrange("p (h d) -> p h d", h=6),
                bvb.rearrange("p (h d) -> p h d", h=6))

        def k_group(at, sbk):
            ps = pj.tile([128, 512], dt.float32, tag="pj", bufs=2, name="pk")
            for dh in range(6):
                nc.tensor.matmul(
                    ps, wk_sb[:, dh, at * 128:(at + 1) * 128],
                    xsb[:, sbk, dh, :], start=(dh == 0), stop=(dh == 5))
                if dh == 2:
                    yield
            nc.vector.tensor_scalar_add(
                kT[:, at, sbk * 512:(sbk + 1) * 512], ps, bk_sb[:, at:at + 1])

        def q_group(at, sb):
            ps = pj.tile([128, 512], dt.float32, tag="pj", bufs=2, name="pq")
            for dh in range(6):
                nc.tensor.matmul(
                    ps, wq_sb[:, dh, at * 128:(at + 1) * 128],
                    xlsb[:, sb, dh, 4:516], start=(dh == 0), stop=(dh == 5))
                if dh == 2:
                    yield
            nc.vector.tensor_scalar_add(
                qT[:, at, sb * 512:(sb + 1) * 512], ps, bq_sb[:, at:at + 1])

        def dw_group(ct, sbc):
            ps = pj.tile([128, 512], dt.float32, tag="pj", bufs=2, name="pdw")
            for k in range(K):
                nc.tensor.matmul(
                    ps, dwd_sb[:, ct, k, :],
                    xlsb[:, sbc, ct, k:k + 512], start=(k == 0), stop=(k == K - 1))
                if k in (2, 5):
                    yield
            nc.vector.tensor_copy(dwT[:, ct, sbc * 512:(sbc + 1) * 512], ps)

        CO_CHUNKS = ((0, 512, 0, 0), (512, 512, 1, 0), (1024, 8, 1, 512))

        def co_group(at, j):
            o, w, sbc, lo = CO_CHUNKS[j]
            ps = pj.tile([128, 512], dt.float32, tag="pj", bufs=2, name="pco")
            for dh in range(6):
                nc.tensor.matmul(
                    ps[:, :w], wco_sb[:, dh, at * 128:(at + 1) * 128],
                    xlsb[:, sbc, dh, lo:lo + w], start=(dh == 0), stop=(dh == 5))
                if dh == 2 and w == 512:
                    yield
            nc.vector.scalar_tensor_tensor(
                out=coT[:, at, o:o + w], in0=ps[:, :w],
                scalar=bco_sb[:, at:at + 1], in1=mask_sb[:, o:o + w],
                op0=Alu.add, op1=Alu.mult)

        def pw_group(at, sbc):
            ps = pj.tile([128, 512], dt.float32, tag="pj", bufs=2, name="ppw")
            for dh in range(6):
                nc.tensor.matmul(
                    ps, wpw_sb[:, dh, at * 128:(at + 1) * 128],
                    dwT[:, dh, sbc * 512:(sbc + 1) * 512],
                    start=(dh == 0), stop=(dh == 5))
                if dh == 2:
                    yield
            nc.vector.tensor_scalar_add(
                kcT[:, at, sbc * 512:(sbc + 1) * 512], ps, convb_sb[:, at:at + 1])

        def ca_mul(at):
            nc.vector.tensor_mul(caT[:, at, :], kcT[:, at, :], qT[:, at, :])
            return
            yield

        def ckl_group(sbc):
            ps = pj.tile([54, 512], dt.float32, tag="pj", bufs=2, name="pckp")
            for at in range(3):
                nc.tensor.matmul(
                    ps, wck_sb[:, at, :], caT[:, at, sbc * 512:(sbc + 1) * 512],
                    start=(at == 0), stop=(at == 2))
            nc.scalar.activation(pck[:, sbc * 512:(sbc + 1) * 512], ps,
                                 Act.Exp, bias=bck_sb, scale=1.0)
            return
            yield

        def pck_store():
            nc.sync.dma_start(out=pck_dram, in_=pck)
            return
            yield

        def pdn_st(st):
            ps = pj.tile([128, 512], dt.float32, tag="pj", bufs=2, name="pdn")
            nc.tensor.matmul(ps[:, 0:6], pck[:, st * 128:(st + 1) * 128], sel_sb,
                             start=True, stop=True)
            nc.vector.reciprocal(recipc[:, st, :], ps[:, 0:6])
            return
            yield

        def wein_k(at, k):
            # two independent accumulation chains (even/odd taps) to halve
            # the serial DVE latency; joined by wein_fin.
            ckb = work.tile([128, CHUNK], dt.bfloat16, tag="ckb", bufs=4,
                            name="ckb")
            for hh in range(2):
                srcap = bass.AP(
                    tensor=pck_dram.tensor,
                    offset=(18 * at + 9 * hh + k) * CHUNK,
                    ap=[[0, 64], [1, CHUNK]])
                nc.sync.dma_start(out=ckb[hh * 64:(hh + 1) * 64], in_=srcap)
            acc = accA if k % 2 == 0 else accB
            if k < 2:
                nc.vector.tensor_mul(acc[:, at, :], ckb, coT[:, at, k:k + CHUNK])
            else:
                tmp = work.tile([128, CHUNK], dt.bfloat16, tag="tmp", bufs=3,
                                name="tmp")
                nc.vector.tensor_mul(tmp, ckb, coT[:, at, k:k + CHUNK])
                nc.vector.tensor_add(acc[:, at, :], acc[:, at, :], tmp)
            return
            yield

        def wein_fin(at):
            nc.vector.tensor_add(accT[:, at, :], accA[:, at, :], accB[:, at, :])
            return
            yield

        def wein_st(at, st):
            # transpose on the DMA xbar (PE stays free for matmuls)
            tps = work.tile([128, 128], dt.bfloat16, tag="tps", bufs=3,
                            name="tps")
            nc.sync.dma_start_transpose(
                tps, accT[:, at, st * 128:(st + 1) * 128])
            for hh in range(2):
                h = at * 2 + hh
                nc.vector.tensor_scalar_mul(
                    stg[:, st, 384 + h * 64: 384 + (h + 1) * 64],
                    tps[:, hh * 64:(hh + 1) * 64], recipc[:, st, h:h + 1])
            return
            yield

        # ---------------- attention (head pairs, sb-outer) ----------------
        filler = deque()

        def pop_filler(n):
            # advance the filler stream by n micro-steps (~3 matmuls each)
            while n > 0 and filler:
                try:
                    next(filler[0])
                except StopIteration:
                    filler.popleft()
                    continue
                n -= 1

        def run_gen(g):
            for _ in g:
                pass

        def attn_block(at, sb, budget):
            cpe = pc.tile([65, 512], dt.float32, tag="cpe", bufs=1, name="cpe")
            cpo = pc.tile([65, 512], dt.float32, tag="cpo", bufs=1, name="cpo")
            for sk in range(16):
                sc = pa.tile([128, 1024], dt.float32, tag="sc", bufs=2,
                             name="sc")
                nc.tensor.matmul(
                    sc[:, 0:512], kT[0:64, at, sk * 128:(sk + 1) * 128],
                    qT[0:64, at, sb * 512:(sb + 1) * 512],
                    start=True, stop=True)
                nc.tensor.matmul(
                    sc[:, 512:1024], kT[64:128, at, sk * 128:(sk + 1) * 128],
                    qT[64:128, at, sb * 512:(sb + 1) * 512],
                    start=True, stop=True)
                pt = work.tile([128, 1024], dt.bfloat16, tag="pt", bufs=4,
                               name="pt")
                nc.scalar.activation(pt, sc, Act.Exp, scale=0.125)
                nc.tensor.matmul(cpe, vsb[:, sk, 2 * at, :], pt[:, 0:512],
                                 start=(sk == 0), stop=(sk == 15))
                nc.tensor.matmul(cpo, vsb[:, sk, 2 * at + 1, :],
                                 pt[:, 512:1024],
                                 start=(sk == 0), stop=(sk == 15))
                pop_filler(budget)
            for hh, cp in ((0, cpe), (1, cpo)):
                h = 2 * at + hh
                cx = work.tile([65, 512], dt.bfloat16, tag="cx", bufs=2,
                               name="cx")
                nc.vector.tensor_copy(cx, cp)
                for s4 in range(4):
                    st = sb * 4 + s4
                    tp = pj.tile([128, 65], dt.bfloat16, tag="pj", bufs=2,
                                 name="tp")
                    nc.tensor.transpose(
                        tp, cx[:, s4 * 128:(s4 + 1) * 128],
                        ident[0:65, 0:65])
                    rcp = work.tile([128, 1], dt.float32, tag="rcp", bufs=4,
                                    name="rcp")
                    nc.vector.reciprocal(rcp, tp[:, 64:65])
                    nc.vector.tensor_scalar_mul(
                        stg[:, st, h * 64:(h + 1) * 64], tp[:, 0:64], rcp)

        def out_st(st):
            nc.sync.dma_start(out=out[:, st * 768:(st + 1) * 768],
                              in_=stg[:, st, :])
            return
            yield

        # ---------------- main emission ----------------
        # phase A: only x-chunk-0 work (all that can run while the rest of
        # the input DMA stream is still in flight); later chunks' consumers
        # go through the filler queue in chunk-arrival order so the PE FIFO
        # never head-of-line blocks on a DMA.
        run_gen(k_group(0, 0))
        run_gen(q_group(0, 0))
        run_gen(v_tile(0))
        run_gen(v_tile(1))

        filler.append(v_tile(2))
        filler.append(v_tile(3))
        filler.append(k_group(0, 1))
        filler.append(q_group(0, 1))
        filler.append(v_tile(4))
        filler.append(v_tile(5))
        filler.append(k_group(0, 2))
        filler.append(v_tile(6))
        filler.append(v_tile(7))
        filler.append(k_group(0, 3))
        for st in range(8, 16):
            filler.append(v_tile(st))
        for sbk in range(4):
            filler.append(k_group(1, sbk))
        filler.append(q_group(1, 0))
        filler.append(q_group(1, 1))
        attn_block(0, 0, budget=2)

        for sbk in range(4):
            filler.append(k_group(2, sbk))
        filler.append(q_group(2, 0))
        filler.append(q_group(2, 1))
        for sbc in range(2):
            for ct in range(6):
                filler.append(dw_group(ct, sbc))
        attn_block(0, 1, budget=2)

        for at in range(3):
            for j in range(3):
                filler.append(co_group(at, j))
        for at in range(3):
            for sbc in range(2):
                filler.append(pw_group(at, sbc))
        attn_block(1, 0, budget=2)

        for at in range(3):
            filler.append(ca_mul(at))
        for sbc in range(2):
            filler.append(ckl_group(sbc))
        filler.append(pck_store())
        for st in range(8):
            filler.append(pdn_st(st))
        for k in range(K):
            for at in range(3):
                filler.append(wein_k(at, k))
        attn_block(1, 1, budget=3)

        for at in range(3):
            filler.append(wein_fin(at))
        for st in range(4):                # st-major so early st complete
            for at in range(3):
                filler.append(wein_st(at, st))
        attn_block(2, 0, budget=3)

        for st in range(4):
            filler.append(out_st(st))
        for st in range(4, 8):
            for at in range(3):
                filler.append(wein_st(at, st))
        attn_block(2, 1, budget=3)

        while filler:
            run_gen(filler.popleft())

        # ---------------- write out (remaining rows) ----------------
        for st in range(4, 8):
            run_gen(out_st(st))

    nc.compile()
    return nc


def _prep_in_maps(inputs):
    x = np.asarray(inputs["x"], np.float32)
    dw = np.asarray(inputs["dw"], np.float32).reshape(HIDDEN, K)

    def sb_layout(wT, ntile):  # [ntile*128, F] -> [128, ntile*F]
        f = wT.shape[1]
        return np.ascontiguousarray(
            wT.reshape(ntile, 128, f).transpose(1, 0, 2).reshape(128, ntile * f))

    def wprep(w):  # [A, HIDDEN] -> bf16 [128, 6*A]
        return sb_layout(np.ascontiguousarray(w.T).astype(BF16), 6)

    com = {
        "wq": wprep(inputs["Wq"]), "wk": wprep(inputs["Wk"]),
        "wv": wprep(inputs["Wv"]), "wco": wprep(inputs["Wco"]),
        "wpw": wprep(inputs["pw"]),
        "wck": sb_layout(np.ascontiguousarray(inputs["Wck"].T).astype(BF16), 3),
        "sel": np.kron(np.eye(N_HEADS), np.ones((K, 1))).astype(BF16),
        "bvrow": inputs["bv"].reshape(1, ALL_HEAD).astype(BF16),
        "bq": np.ascontiguousarray(inputs["bq"].reshape(3, 128).T, np.float32),
        "bk": np.ascontiguousarray(inputs["bk"].reshape(3, 128).T, np.float32),
        "convb": np.ascontiguousarray(
            inputs["conv_bias"].reshape(3, 128).T, np.float32),
        "bco": np.ascontiguousarray(inputs["bco"].reshape(3, 128).T, np.float32),
        "bck": inputs["bck"].reshape(54, 1).astype(np.float32),
    }
    # depthwise taps in SBUF layout [c_within, ct, k]; diagonal matrices are
    # built on-device from these
    com["dwv"] = np.ascontiguousarray(
        dw.reshape(6, 128, K).transpose(1, 0, 2).reshape(128, 6 * K)).astype(np.float32)

    in_maps = []
    for b in range(B):
        xb = x[b]                                   # [S, HIDDEN]
        xTb = np.ascontiguousarray(xb.T).astype(BF16)   # [768, S]
        # full-seq x in 4 chunks of 512 cols
        x4 = np.stack([
            sb_layout(np.ascontiguousarray(xTb[:, c * 512:(c + 1) * 512]), 6)
            for c in range(4)], axis=1)             # [128, 4, 6*512]
        x4 = np.ascontiguousarray(x4.reshape(128, 4 * 6 * 512))
        for j in range(2):
            # local chunk with +-4 halo, split into 2 overlapping halves of 520
            loc = np.zeros((HIDDEN, 1032), BF16)
            g0 = j * CHUNK - 4
            lo_src = max(0, g0)
            hi_src = min(S, g0 + 1032)
            loc[:, lo_src - g0:hi_src - g0] = xTb[:, lo_src:hi_src]
            xl2 = np.stack([
                sb_layout(np.ascontiguousarray(loc[:, 0:520]), 6),
                sb_layout(np.ascontiguousarray(loc[:, 512:1032]), 6)], axis=1)
            xl2 = np.ascontiguousarray(xl2.reshape(128, 2 * 6 * 520))
            mrows = np.arange(g0, g0 + 1032)
            comask = ((mrows >= 0) & (mrows < S)).astype(BF16).reshape(1, 1032)
            m = dict(com)
            m["x4"] = x4
            m["xl2"] = xl2
            m["comask"] = comask
            in_maps.append(m)
    return in_maps


def _gather(results):
    # per-core out: [128, 8*768] where row s_local = st*128 + p
    outs = []
    for r in results:
        o = np.asarray(r["out"], np.float32).reshape(128, 8, 768)
        outs.append(np.ascontiguousarray(o.transpose(1, 0, 2)).reshape(1024, 768))
    full = np.stack(outs).reshape(B, 2, CHUNK, 768).reshape(B, S, 768)
    return full


def kernel(**inputs):
    from concourse.bass_utils import run_bass_kernel_spmd

    key = "prog"
    if key not in _COMPILED:
        _COMPILED[key] = _build_program()
    nc = _COMPILED[key]
    in_maps = _prep_in_maps(inputs)
    res = run_bass_kernel_spmd(nc, in_maps, list(range(N_CORES)))
    return _gather(res.results)


if __name__ == "__main__":
    import reference
    inp = {k: np.asarray(v) for k, v in reference.setup_inputs().items()}
    got = kernel(**inp)
    want = np.asarray(reference.reference(**inp))
    err = np.linalg.norm(got - want) / np.linalg.norm(want)
    print("rel err:", err)
